# revision 2
# baseline (speedup 1.0000x reference)
"""Trainium2 Bass kernel for the 4-branch "Memory multimode" attention module.

Reference computation (per batch element b):
    q  = q_key[b].reshape(1024, 128)        (row-major reinterpret)
    pq = p_q_key[b].reshape(1024, 128)
    k  = m_key[b].reshape(128, 2048)
    pk = p_m_key[b].reshape(128, 2048)
    mval = m_val[b].reshape(512, 2048).T    # [2048, 512]
    out  = (sm(q@k) + sm(pq@pk) + sm(pq@k) + sm(q@pk)) @ mval
    where sm() is softmax over the QUERY dim (axis 0 of each [1024, 2048] score
    matrix).  Final output channel-concats q_val.

Key algebraic point: all four branches share the same value matrix, so the four
softmax matrices are summed BEFORE the value matmul - one [1024,2048]@[2048,512]
matmul instead of four (2.5x FLOP reduction vs the naive form).

Implementation (one NeuronCore per batch element, 8 cores, data-parallel):
  * Transposed score layout S^T = [key_pos(l) x query(i)]: softmax reduction
    runs along the free dim; S^T tiles come straight off the PE with
    lhsT = keys l-tile (natural layout) and rhs = Q^T (host pre-transposed).
  * Single-pass fp16 score matmuls (1 cyc/row on the PE, ~11-bit operand
    mantissa).  The correctness budget (rel err < 2e-2 vs max|out|) has >10x
    headroom over the ~1.4e-3 this costs end-to-end.
  * No max-subtraction needed: |scores| <= ~75, exp stays in fp32/bf16
    exponent range.  ScalarE exp emits bf16 E tiles (bf16 keeps the fp32
    exponent, so no overflow) plus fused row-sum denominators via accum_out.
    The exp sweep over 8M scores is the bottleneck engine (~83us busy);
    everything else is arranged to hide under it.
  * The 1/D scaling + 4-branch sum runs on the VectorE as a
    tensor_scalar / scalar_tensor_tensor chain in 16-bit (2x DVE mode),
    writing A^T to SBUF as fp16 for the value matmul.
  * Value matmul in fp16 (1 cyc/row); n_overlap of its 8 output-row PSUM
    accumulations are interleaved into phase 1 using the PSUM banks left
    free by the score tiles.  Value matmuls lag one l-tile behind the score
    matmuls in the PE queue so the (strict-FIFO) PE never stalls waiting on
    the DVE chain.
  * Output staged to SBUF as fp16 by the DVE (ScalarE is the bottleneck and
    stays exp-only); host upcasts and concatenates q_val.
"""

import numpy as np

import concourse.bass as bass
import concourse.mybir as mybir
import concourse.tile as tile
from concourse.bass_utils import run_bass_kernel_spmd
from concourse.vector_clock import ScopedClock

# The walrus build in this image supports only ONE sync-wait command per
# instruction (CTRL_NO_STRUCT / S3_LW_STRUCT encodings); this concourse's Tile
# scheduler freely attaches several.  Two fixes: (1) split the kernel-tail
# drain's waits over several drains, (2) a post-scheduling pass that moves
# overflow waits onto NoOps inserted before the over-subscribed instruction.
_MAX_WAITS = 1


def _split_drain_and_barrier(self, tick_clock, wait_clock):
    nc = self.nc
    drain_inst = nc.sync.drain()
    wait_clock.add_sem_waits(
        drain_inst.ins, ScopedClock({None: tick_clock.global_clock})
    )
    mi = drain_inst.ins
    waits = list(mi.sync_info.on_wait)
    if len(waits) > _MAX_WAITS:
        del mi.sync_info.on_wait[_MAX_WAITS:]
        rest = waits[_MAX_WAITS:]
        for i in range(0, len(rest), _MAX_WAITS):
            extra = nc.sync.drain()
            if extra.ins.sync_info is None:
                extra.ins.sync_info = mybir.SyncInfo(on_wait=[], on_update=[])
            extra.ins.sync_info.on_wait.extend(rest[i : i + _MAX_WAITS])

    nc.all_engine_barrier()
    assert self.sems is not None
    popped = nc._tile_sem_poison_stack.pop()
    assert popped is self._sem_poison
    nc.clear_and_free_semaphores(list(self.sems.allocated().values()))
    nc.all_engine_barrier()


tile.TileContext._drain_and_barrier = _split_drain_and_barrier


def _split_sync_waits(nc, cap: int = _MAX_WAITS):
    for f in nc.m.functions:
        for blk in f.blocks:
            out = []
            changed = False
            for inst in blk.instructions:
                si = inst.sync_info
                if si is not None and len(si.on_wait) > cap:
                    waits = list(si.on_wait)
                    rest, keep = waits[:-cap], waits[-cap:]
                    for i in range(0, len(rest), cap):
                        noop = mybir.InstNoOp(
                            name=nc.get_next_instruction_name(), ins=[], outs=[]
                        )
                        noop.engine = inst.engine
                        noop.sync_info = mybir.SyncInfo(
                            on_wait=rest[i : i + cap], on_update=[]
                        )
                        nc.register_instruction(noop)
                        out.append(noop)
                    inst.sync_info = mybir.SyncInfo(
                        on_wait=keep, on_update=list(si.on_update)
                    )
                    changed = True
                out.append(inst)
            if changed:
                blk.instructions = out
    return nc


B, H, W = 8, 32, 32
HW = H * W          # 1024 queries
KD = 128            # key dim
VD = 512            # val dim
L = 2 * HW          # 2048 key positions per key matrix
NT = L // 128       # 16 l-tiles
NO = HW // 128      # 8 output row-tiles
NCORES = 8

F32 = mybir.dt.float32
BF16 = mybir.dt.bfloat16
F16 = mybir.dt.float16

_nc_cache = {}


def build_nc(n_overlap: int = 4):
    """n_overlap: output-row PSUM accumulations interleaved into phase 1
    (each holds one PSUM bank for the whole phase; score tiles use 4)."""
    nc = bass.Bass("TRN2", target_bir_lowering=False, debug=False)

    def din(name, shape, dt):
        return nc.dram_tensor(name, shape, dt, kind="ExternalInput").ap()

    kt_d = din("kt", [KD, 2 * L], F16)    # m_key | p_m_key, fp16
    qt_d = din("qt", [KD, 2 * HW], F16)   # q^T | pq^T, fp16
    mvt_d = din("mvt", [L, VD], F16)      # m_val reinterpreted+transposed, fp16
    out_d = nc.dram_tensor("out", [HW, VD], F16, kind="ExternalOutput").ap()

    EXP = mybir.ActivationFunctionType.Exp
    MUL = mybir.AluOpType.mult
    ADD = mybir.AluOpType.add

    with tile.TileContext(nc) as tc:
        with (
            tc.tile_pool(name="keys", bufs=1) as keys_pool,
            tc.tile_pool(name="qts", bufs=1) as qt_pool,
            tc.tile_pool(name="mv", bufs=1) as mv_pool,
            tc.tile_pool(name="ework", bufs=2) as e_pool,
            tc.tile_pool(name="atiles", bufs=1) as a_pool,
            tc.tile_pool(name="dwork", bufs=2) as d_pool,
            tc.tile_pool(name="ostage", bufs=2) as out_pool,
            tc.tile_pool(name="psum_s", bufs=2, space="PSUM") as psum_s,
            tc.tile_pool(name="psum_o", bufs=1, space="PSUM") as psum_o,
        ):
            # ---- input loads, ordered so the first score matmuls start early:
            # queries first, then keys (first l-half of both key tensors
            # before the second halves), value tiles last.
            qt = qt_pool.tile([128, 2 * HW], F16, tag="qt")
            nc.sync.dma_start(qt[:], qt_d)
            kt = keys_pool.tile([128, 2 * L], F16, tag="kt")
            for half in range(2):
                for y in range(2):
                    sl = slice(y * L + half * (L // 2), y * L + (half + 1) * (L // 2))
                    nc.sync.dma_start(kt[:, sl], kt_d[:, sl])
            mv = mv_pool.tile([128, NT * VD], F16, tag="mv")
            for t in range(NT):
                nc.sync.dma_start(
                    mv[:, t * VD : (t + 1) * VD], mvt_d[t * 128 : (t + 1) * 128, :]
                )

            # phase-1-resident output accumulators (one PSUM bank each)
            o_acc = [
                psum_o.tile([128, VD], F32, tag=f"O{i}", name=f"o_acc{i}")
                for i in range(n_overlap)
            ]

            # ---- phase 1 ---------------------------------------------------
            a_tiles = []

            def value_mms(t):
                # interleaved value-matmul accumulation for the first rows
                for i in range(n_overlap):
                    nc.tensor.matmul(
                        o_acc[i][:],
                        a_tiles[t][:, i * 128 : (i + 1) * 128],
                        mv[:, t * VD : (t + 1) * VD],
                        start=(t == 0),
                        stop=(t == NT - 1),
                    )

            for t in range(NT):
                dtile = d_pool.tile([128, 4], F32, tag="D")
                e_tiles = []
                for y in range(2):
                    for xh in range(2):
                        br = 2 * y + xh
                        s_ps = psum_s.tile([128, HW], F32, tag="S")
                        for c in range(2):
                            nc.tensor.matmul(
                                s_ps[:, c * 512 : (c + 1) * 512],
                                kt[:, y * L + t * 128 : y * L + (t + 1) * 128],
                                qt[:, xh * HW + c * 512 : xh * HW + (c + 1) * 512],
                                start=True, stop=True)
                        # E^T = exp(S^T) in bf16; accum_out = row sum = denom
                        e_t = e_pool.tile([128, HW], BF16, tag=f"E{br}")
                        nc.scalar.activation(
                            e_t[:], s_ps[:], EXP,
                            accum_out=dtile[:, br : br + 1],
                        )
                        e_tiles.append(e_t)
                    # value matmuls for tile t-1 go after tile t's first
                    # branch pair: their A^T is ready by now, and the PE
                    # (strict FIFO) never stalls the score stream.
                    if y == 0 and t > 0:
                        value_mms(t - 1)

                invd = d_pool.tile([128, 4], F32, tag="invD")
                nc.vector.reciprocal(invd[:], dtile[:])

                # A^T[t] = sum_br invD_br * E_br  (DVE, 16-bit 2x modes)
                a_sb = a_pool.tile([128, HW], F16, tag=f"A{t}")
                tmp0 = d_pool.tile([128, HW], F16, tag="t0")
                nc.vector.tensor_scalar_mul(tmp0[:], e_tiles[0][:], invd[:, 0:1])
                tmp1 = d_pool.tile([128, HW], F16, tag="t1")
                nc.vector.scalar_tensor_tensor(
                    tmp1[:], e_tiles[1][:], invd[:, 1:2], tmp0[:], MUL, ADD)
                tmp2 = d_pool.tile([128, HW], F16, tag="t2")
                nc.vector.scalar_tensor_tensor(
                    tmp2[:], e_tiles[2][:], invd[:, 2:3], tmp1[:], MUL, ADD)
                nc.vector.scalar_tensor_tensor(
                    a_sb[:], e_tiles[3][:], invd[:, 3:4], tmp2[:], MUL, ADD)
                a_tiles.append(a_sb)

            value_mms(NT - 1)

            # ---- phase 2: drain overlapped rows, then the remaining rows ---
            for i in range(NO):
                if i < n_overlap:
                    o_ps = o_acc[i]
                else:
                    o_ps = psum_s.tile([128, VD], F32, tag="S",
                                       name=f"o_tail{i}")
                    for t in range(NT):
                        nc.tensor.matmul(
                            o_ps[:],
                            a_tiles[t][:, i * 128 : (i + 1) * 128],
                            mv[:, t * VD : (t + 1) * VD],
                            start=(t == 0),
                            stop=(t == NT - 1),
                        )
                o_sb = out_pool.tile([128, VD], F16, tag="osb")
                # DVE stages (and downcasts) the output; ScalarE stays
                # exp-only and the PE tail hides these copies.
                nc.vector.tensor_copy(o_sb[:], o_ps[:])
                nc.sync.dma_start(out_d[i * 128 : (i + 1) * 128, :], o_sb[:])

    _split_sync_waits(nc)
    return nc


def make_in_maps(m_key, m_val, q_key, p_m_key, p_q_key):
    in_maps = []
    for b in range(B):
        kt = np.empty((KD, 2 * L), np.float16)
        kt[:, :L] = m_key[b].reshape(KD, L)
        kt[:, L:] = p_m_key[b].reshape(KD, L)
        qt = np.empty((KD, 2 * HW), np.float16)
        qt[:, :HW] = q_key[b].reshape(HW, KD).T
        qt[:, HW:] = p_q_key[b].reshape(HW, KD).T
        mvt = np.ascontiguousarray(
            m_val[b].reshape(VD, L).T.astype(np.float16))
        in_maps.append({"kt": kt, "qt": qt, "mvt": mvt})
    return in_maps


def run(inputs, trace: bool = False, n_overlap: int = 4):
    """Run on the 8 NeuronCores; returns (full_output, BassKernelResults)."""
    inputs = {k: np.asarray(v, dtype=np.float32) for k, v in inputs.items()}
    if n_overlap not in _nc_cache:
        _nc_cache[n_overlap] = build_nc(n_overlap)
    nc = _nc_cache[n_overlap]
    in_maps = make_in_maps(
        inputs["m_key"], inputs["m_val"], inputs["q_key"],
        inputs["p_m_key"], inputs["p_q_key"],
    )
    res = run_bass_kernel_spmd(nc, in_maps, list(range(NCORES)), trace=trace)
    q_val = inputs["q_val"]
    outs = []
    for b in range(B):
        mat = np.asarray(res.results[b]["out"]).astype(np.float32)
        attn = mat.reshape(VD, H, W)                 # reinterpret, no transpose
        outs.append(np.concatenate([attn, q_val[b]], axis=0))
    return np.stack(outs), res


def kernel(**inputs) -> np.ndarray:
    out, _ = run(inputs, trace=False)
    return out
